# revision 3
# baseline (speedup 1.0000x reference)
"""CorefScore kernel for 8 Trainium2 NeuronCores — h-major redesign.

Layout: PSUM tiles are [128 pairs, 152] (pair-major); pairs = (delta, m)
with m minor. Per pair-block: 8 product matmuls (stationary = DVE-formed
shifted products P[d, pair], moving = W1c tile [128, 150]) + 3 C-add
passes (shifted-identity slices of a host EE tensor against Ya / Yb
chunk tiles). The w2p contraction is done on ScalarE via relu+accum_out
over sign-reordered, |w2p|-prescaled hidden columns (neg group first),
so no transposes or respreads are needed: accumulator columns land
directly in [m, delta] order. ment_j and ment_i ride along as carrier
columns 150/151 (offset +CARRIER to survive the relu). b1p/b1m/carrier
constants enter via an all-ones row 900 of the padded X^T. Products for
odd deltas read a 1-element-shifted copy of X^T so every DVE op is
4B-aligned (2x_1p DVE mode).
"""

import os
import sys

import numpy as np

for _p in ("/opt/trn_rl_repo", "/opt/pypackages"):
    if os.path.isdir(_p) and _p not in sys.path:
        sys.path.append(_p)

import concourse.bacc as bacc
import concourse.mybir as mybir
import concourse.tile as tile
from concourse.ap import AP
from concourse.bass_utils import run_bass_kernel_spmd

F16 = mybir.dt.float16
F32 = mybir.dt.float32
AF = mybir.ActivationFunctionType
ALU = mybir.AluOpType

M, D, H, K = 2048, 900, 150, 50
NCORES = 8
MC = M // NCORES          # 256 owned mentions per core
HALO = 64                 # halo columns before owned range
WP = 384                  # padded window width (owned 256 + halo 64 + pad 64)
DP = 1024                 # padded feature dim
NDT = DP // 128           # 8 d tiles
CW = 151                  # acts 150 + carrier col (ment_j + CARRIER)
CARRIER = 100.0
ONES_ROW = 900            # all-ones row of padded X^T (bias carrier)

# delta groups: same parity batched so the DVE shifted read keeps 4B
# alignment (stride -2 over the delta axis)
_EV = list(range(2, K + 1, 2))
_OD = list(range(1, K + 1, 2))
def _chunks(xs, sizes):
    out = []
    i = 0
    for s in sizes:
        out.append(xs[i:i + s])
        i += s
    return out

_SZ = (7, 6, 6, 6)
GROUPS = [(0, g) for g in _chunks(_EV, _SZ)] + \
         [(1, g) for g in _chunks(_OD, _SZ)]

_cache = {}
DBG = False


def _ap3(t_ap, p_lo, p_n, off, dims):
    """3-D free-dim view of a tile AP."""
    b = t_ap[p_lo:p_lo + p_n, 0:1]
    pstride = b.ap[0][0]
    return AP(b.tensor, b.offset + off, [[pstride, p_n]] + [list(d) for d in dims])


def _dap(base_ap, off, dims):
    """Free-form AP on a DRAM tensor."""
    return AP(base_ap.tensor, base_ap.offset + off, [list(d) for d in dims])


def _build(nn, nm):
    """nn: count of negative w2p columns (neg group first); nm: same for w2m."""
    nc = bacc.Bacc("TRN2", target_bir_lowering=False, debug=False)

    xt2_d = nc.dram_tensor("xt2", [128, 2 * NDT * WP], F16,
                           kind="ExternalInput").ap()
    wall_d = nc.dram_tensor("wall", [128, 4 * NDT * CW], F16,
                            kind="ExternalInput").ap()
    ee_d = nc.dram_tensor("ee", [128, 320], F16, kind="ExternalInput").ap()
    mask_d = nc.dram_tensor("mask", [128, 4 * (K + 1)], F32,
                            kind="ExternalInput").ap()
    out_d = nc.dram_tensor("out", [MC, K + 1], F32, kind="ExternalOutput").ap()
    ybdram = nc.dram_tensor("ybdram", [384, CW], F16, kind="Internal").ap()
    if DBG:
        dya_d = nc.dram_tensor("dya", [128, CW], F16, kind="ExternalOutput").ap()
        dyb_d = nc.dram_tensor("dyb", [128, 3 * CW], F16, kind="ExternalOutput").ap()
        dpos_d = nc.dram_tensor("dpos", [128, K], F32, kind="ExternalOutput").ap()
        dneg_d = nc.dram_tensor("dneg", [128, K], F32, kind="ExternalOutput").ap()
        dmi_d = nc.dram_tensor("dmi", [128, 1], F32, kind="ExternalOutput").ap()
        dps_d = nc.dram_tensor("dps", [128, CW], F32, kind="ExternalOutput").ap()
        dpr_d = nc.dram_tensor("dpr", [128, MC], F16, kind="ExternalOutput").ap()

    with tile.TileContext(nc) as tc:
        with (
            tc.tile_pool(name="const", bufs=1) as cp,
            tc.tile_pool(name="prod", bufs=3) as prp,
            tc.tile_pool(name="work", bufs=2) as wp,
            tc.tile_pool(name="ps_pre", bufs=2, space="PSUM") as pp_pre,
            tc.tile_pool(name="ps_blk", bufs=6, space="PSUM") as pp_blk,
        ):
            # ---- input DMAs (big merged transfers) ----
            wall = cp.tile([128, 4 * NDT * CW], F16, tag="wall")
            nc.scalar.dma_start(out=wall[:], in_=wall_d[:])
            ee = cp.tile([128, 320], F16, tag="ee")
            nc.scalar.dma_start(out=ee[:], in_=ee_d[:])
            mask = cp.tile([128, 4 * (K + 1)], F32, tag="mask")
            nc.scalar.dma_start(out=mask[:], in_=mask_d[:])
            xt2 = cp.tile([128, 2 * NDT * WP], F16, tag="xt2")
            Q = NDT * WP // 2
            for q in range(4):
                nc.sync.dma_start(out=xt2[:, Q * q:Q * (q + 1)],
                                  in_=xt2_d[:, Q * q:Q * (q + 1)])
            xt = xt2[:, 0:NDT * WP]
            xto = xt2[:, NDT * WP:2 * NDT * WP]
            WB = NDT * CW
            w1m, w1a, w1b, w1c = (wall[:, i * WB:(i + 1) * WB] for i in range(4))
            mm_sb = [mask[:, 0:K + 1], mask[:, 2 * (K + 1):3 * (K + 1)]]
            ma_sb = [mask[:, K + 1:2 * (K + 1)], mask[:, 3 * (K + 1):4 * (K + 1)]]

            # ---- HAM warm-up: dummy matmuls on garbage while inputs load ----
            wscr = cp.tile([128, 512], F16, tag="wscr")
            nc.gpsimd.memset(wscr[:], 0.5)
            for i in range(17):
                ps = pp_blk.tile([128, CW], F32, tag="blk")
                nc.tensor.matmul(ps[:], wscr[:, 0:128], wscr[:, 0:CW],
                                 start=True, stop=True)

            def xt_sl(t, c0, n, odd=False):
                src = xto if odd else xt
                return src[:, WP * t + c0: WP * t + c0 + n]

            def w_sl(w, t, n=CW):
                return w[:, CW * t: CW * t + n]

            # identity and shifted-identity slices of EE ([0 | I | 0]):
            # EE[:, 128+s : 256+s] has [c, p] = 1 iff c == p + s
            def idsl(s):
                return ee[:, 128 + s: 256 + s]

            # ---- DVE: products for a delta group ----
            def emit_products(gi):
                parity, deltas = GROUPS[gi]
                g = len(deltas)
                d0 = deltas[0]
                tiles = []
                for t in range(NDT):
                    pr = prp.tile([128, 7 * MC], F16, tag=f"pr{t}")
                    base = WP * t + HALO
                    off = base - d0 - parity  # even element offset by construction
                    src = xto if parity else xt
                    nc.vector.tensor_tensor(
                        _ap3(pr[:], 0, 128, 0, [(MC, g), (1, MC)]),
                        _ap3(xt[:], 0, 128, base, [(0, g), (1, MC)]),
                        _ap3(src[:], 0, 128, off, [(-2, g), (1, MC)]),
                        ALU.mult)
                    tiles.append(pr)
                return tiles

            prod_q = [emit_products(0), emit_products(1), emit_products(2)]

            # ---- PE preamble ----
            # mention-score acts over 3 window chunks
            actm_ps = []
            for c in range(3):
                ps = pp_pre.tile([128, 150], F32, tag="pre")
                for t in range(NDT):
                    nc.tensor.matmul(ps[:], xt_sl(t, 128 * c, 128),
                                     w_sl(w1m, t, 150),
                                     start=(t == 0), stop=(t == NDT - 1))
                actm_ps.append(ps)
            # Ya over the 2 owned blocks (one concatenated tile)
            ya2 = cp.tile([128, 2 * CW], F16, tag="ya2")
            for b in range(2):
                ps = pp_pre.tile([128, CW], F32, tag="pre")
                for t in range(NDT):
                    nc.tensor.matmul(ps[:], xt_sl(t, HALO + 128 * b, 128),
                                     w_sl(w1a, t),
                                     start=(t == 0), stop=(t == NDT - 1))
                nc.scalar.copy(ya2[:, CW * b:CW * (b + 1)], ps[:])
            # Yb over 3 window chunks (one concatenated tile)
            ybe = cp.tile([128, 3 * CW], F16, tag="ybe")
            for c in range(3):
                ps = pp_pre.tile([128, CW], F32, tag="pre")
                for t in range(NDT):
                    nc.tensor.matmul(ps[:], xt_sl(t, 128 * c, 128),
                                     w_sl(w1b, t),
                                     start=(t == 0), stop=(t == NDT - 1))
                nc.scalar.copy(ybe[:, CW * c:CW * (c + 1)], ps[:])

            # ment accum: per chunk, neg/pos group sums -> mcol f32 [128, 3]
            mneg = cp.tile([128, 3], F32, tag="mneg")
            mpos = cp.tile([128, 3], F32, tag="mpos")
            mact = wp.tile([128, 150], F16, tag="mact")
            for c in range(3):
                nc.scalar.activation(mact[:, 0:nm], actm_ps[c][:, 0:nm], AF.Relu,
                                     accum_out=mneg[:, c:c + 1])
                nc.scalar.activation(mact[:, nm:150], actm_ps[c][:, nm:150], AF.Relu,
                                     accum_out=mpos[:, c:c + 1])
            # ment columns: yb[c][:, 150] = mpos - mneg  (on ScalarE:
            # Identity(-mneg + bias=mpos) keeps the chain on one engine)
            mcol = cp.tile([128, 3], F16, tag="mcol")
            for c in range(3):
                nc.scalar.activation(mcol[:, c:c + 1], mneg[:, c:c + 1],
                                     AF.Identity, bias=mpos[:, c:c + 1],
                                     scale=-1.0)
                nc.scalar.activation(ybe[:, CW * c + 150:CW * c + 151],
                                     mneg[:, c:c + 1], AF.Identity,
                                     bias=mpos[:, c:c + 1], scale=-1.0)

            # dump Yb (with ment col) to DRAM; then per-delta shifted windows
            # come back as single affine DMAs (partition shifts are free in
            # linear DRAM)
            nc.sync.dma_start(
                out=_dap(ybdram, 0, [(CW, 128), (128 * CW, 3), (1, CW)]),
                in_=_ap3(ybe[:], 0, 128, 0, [(CW, 3), (1, CW)]))
            ybsh = {}
            for gi in range(2, len(GROUPS)):
                for delta in GROUPS[gi][1]:
                    t = cp.tile([128, 2 * CW], F16, tag=f"ybsh{delta}",
                                name=f"ybsh{delta}")
                    nc.sync.dma_start(
                        out=_ap3(t[:], 0, 128, 0, [(CW, 2), (1, CW)]),
                        in_=_dap(ybdram, (HALO - delta) * CW,
                                 [(CW, 128), (128 * CW, 2), (1, CW)]))
                    ybsh[delta] = t

            # ment_i columns for assembly (f32, exact)
            micol = []
            for b in range(2):
                ps = pp_pre.tile([128, CW], F32, tag="pre")
                nc.tensor.matmul(ps[:, 0:1], idsl(64), mcol[:, b:b + 1],
                                 start=True, stop=False)
                nc.tensor.matmul(ps[:, 0:1], idsl(-64), mcol[:, b + 1:b + 2],
                                 start=False, stop=True)
                mi = cp.tile([128, 1], F32, tag=f"mi{b}")
                nc.scalar.copy(mi[:], ps[:, 0:1])
                micol.append(mi)

            # ---- block loop ----
            pos_t = [cp.tile([128, K], F32, tag=f"pos{b}", name=f"pos{b}")
                     for b in range(2)]
            neg_t = [cp.tile([128, K], F32, tag=f"neg{b}", name=f"neg{b}")
                     for b in range(2)]
            cds = {}

            def emit_cbuild(gi):
                for delta in GROUPS[gi][1]:
                    cd = cp.tile([128, 2 * CW], F16, tag=f"cd{delta}",
                                 name=f"cd{delta}")
                    nc.vector.tensor_tensor(cd[:], ya2[:], ybsh[delta][:],
                                            ALU.add)
                    cds[delta] = cd

            emit_cbuild(2)
            for gi in range(len(GROUPS)):
                parity, deltas = GROUPS[gi]
                prods = prod_q.pop(0)
                if gi + 1 >= 2 and gi + 1 < len(GROUPS):
                    emit_cbuild(gi + 1)
                for di, delta in enumerate(deltas):
                    sa = HALO - delta  # in-chunk start row of the j window
                    for b in range(2):
                        ps = pp_blk.tile([128, CW], F32, tag="blk")
                        for t in range(NDT):
                            nc.tensor.matmul(
                                ps[:],
                                prods[t][:, MC * di + 128 * b: MC * di + 128 * b + 128],
                                w_sl(w1c, t),
                                start=(t == 0), stop=False)
                        # C add: single pass from the prebuilt C tile, or
                        # the 3-pass shifted-identity path for early groups
                        if gi >= 2:
                            nc.tensor.matmul(
                                ps[:], idsl(0),
                                cds[delta][:, CW * b:CW * (b + 1)],
                                start=False, stop=True)
                        else:
                            nc.tensor.matmul(ps[:], idsl(0),
                                             ya2[:, CW * b:CW * (b + 1)],
                                             start=False, stop=False)
                            nc.tensor.matmul(ps[:], idsl(sa),
                                             ybe[:, CW * b:CW * (b + 1)],
                                             start=False, stop=False)
                            nc.tensor.matmul(ps[:], idsl(sa - 128),
                                             ybe[:, CW * (b + 1):CW * (b + 2)],
                                             start=False, stop=True)
                        if DBG and delta == 2 and b == 0:
                            dps_t = wp.tile([128, CW], F32, tag="dps")
                            nc.scalar.copy(dps_t[:], ps[:])
                            nc.sync.dma_start(out=dps_d[:], in_=dps_t[:])
                            nc.sync.dma_start(out=dpr_d[:],
                                              in_=prods[0][:, 0:MC])
                        # evac: relu + accum into per-delta columns
                        kcol = K - delta
                        nc.scalar.activation(ps[:, 0:nn], ps[:, 0:nn], AF.Relu,
                                             accum_out=neg_t[b][:, kcol:kcol + 1])
                        nc.scalar.activation(ps[:, nn:CW], ps[:, nn:CW], AF.Relu,
                                             accum_out=pos_t[b][:, kcol:kcol + 1])
                if gi + 3 < len(GROUPS):
                    prod_q.append(emit_products(gi + 3))

            # ---- assembly ----
            for b in range(2):
                sc = wp.tile([128, K + 1], F32, tag=f"sc{b}")
                nc.vector.memset(sc[:, K:K + 1], 0.0)
                nc.vector.tensor_tensor(sc[:, 0:K], pos_t[b][:], neg_t[b][:],
                                        ALU.subtract)
                nc.vector.tensor_tensor(
                    sc[:, 0:K], sc[:, 0:K],
                    _ap3(micol[b][:], 0, 128, 0, [(0, K)]), ALU.add)
                nc.vector.tensor_tensor(sc[:], sc[:], mm_sb[b], ALU.mult)
                nc.vector.tensor_tensor(sc[:], sc[:], ma_sb[b], ALU.add)
                nc.sync.dma_start(out=out_d[128 * b:128 * (b + 1), :], in_=sc[:])
            if DBG:
                nc.sync.dma_start(out=dya_d[:], in_=ya2[:, 0:CW])
                for c in range(3):
                    nc.sync.dma_start(out=dyb_d[:, CW * c:CW * (c + 1)],
                                      in_=ybe[:, CW * c:CW * (c + 1)])
                nc.sync.dma_start(out=dpos_d[:], in_=pos_t[0][:])
                nc.sync.dma_start(out=dneg_d[:], in_=neg_t[0][:])
                nc.sync.dma_start(out=dmi_d[:], in_=micol[0][:])

    nc.compile()
    return nc


def _prep_inputs(inputs):
    X = np.asarray(inputs["mention_reprs"], dtype=np.float32)
    assert X.shape == (M, D)
    w1p = np.asarray(inputs["w1p"], dtype=np.float32)
    W1A, W1B, W1C = w1p[:D], w1p[D:2 * D], w1p[2 * D:]
    w2p = np.asarray(inputs["w2p"], dtype=np.float32).reshape(H)
    w2m = np.asarray(inputs["w2m"], dtype=np.float32).reshape(H)
    b1p = np.asarray(inputs["b1p"], dtype=np.float32).reshape(H)
    b1m = np.asarray(inputs["b1m"], dtype=np.float32).reshape(H)
    b2p = float(np.asarray(inputs["b2p"]).reshape(-1)[0])
    b2m = float(np.asarray(inputs["b2m"]).reshape(-1)[0])

    # sign-reorder (neg first) and |w2| prescale
    permp = np.argsort(w2p >= 0, kind="stable")
    nn = int((w2p < 0).sum())
    sp = np.abs(w2p)[permp]
    permm = np.argsort(w2m >= 0, kind="stable")
    nm = int((w2m < 0).sum())
    sm = np.abs(w2m)[permm]

    def wext(W, brow, extra150=0.0):
        out = np.zeros((DP, CW), dtype=np.float32)
        out[:D, 0:H] = W[:, permp] * sp[None, :]
        out[ONES_ROW, 0:H] = brow * sp if brow is not None else 0.0
        out[ONES_ROW, 150] = extra150
        return out

    w1a_ext = wext(W1A, b1p[permp])
    w1b_ext = wext(W1B, None)
    w1c_ext = wext(W1C, None, extra150=CARRIER)
    w1m_ext = np.zeros((DP, CW), dtype=np.float32)
    w1m_ext[:D, 0:H] = np.asarray(inputs["w1m"], np.float32)[:, permm] * sm[None, :]
    w1m_ext[ONES_ROW, 0:H] = b1m[permm] * sm

    def pack(Wx):  # [DP, CW] -> [128, NDT*CW] f16
        out = np.empty((128, NDT * CW), dtype=np.float16)
        for t in range(NDT):
            out[:, CW * t:CW * (t + 1)] = Wx[128 * t:128 * (t + 1), :]
        return np.ascontiguousarray(out)

    shared = {
        "wall": np.ascontiguousarray(np.concatenate(
            [pack(w1m_ext), pack(w1a_ext), pack(w1b_ext), pack(w1c_ext)],
            axis=1)),
        "ee": np.ascontiguousarray(np.concatenate(
            [np.zeros((128, 128), np.float16), np.eye(128, dtype=np.float16),
             np.zeros((128, 64), np.float16)], axis=1)),
    }

    XTpad = np.zeros((DP, M + 2 * HALO + 2), dtype=np.float32)
    XTpad[:D, HALO:HALO + M] = X.T
    XTpad[ONES_ROW, :] = 1.0

    in_maps = []
    const_add = np.float32(b2p + 2.0 * b2m - CARRIER)
    for core in range(NCORES):
        r0 = MC * core
        win = XTpad[:, r0:r0 + WP + 1]  # window cols r0-64 .. r0+321 (clipped)
        if win.shape[1] < WP + 1:
            win = np.pad(win, ((0, 0), (0, WP + 1 - win.shape[1])))
        xtp = np.empty((128, NDT * WP), dtype=np.float16)
        xtop = np.empty((128, NDT * WP), dtype=np.float16)
        for t in range(NDT):
            xtp[:, WP * t:WP * (t + 1)] = win[128 * t:128 * (t + 1), 0:WP]
            xtop[:, WP * t:WP * (t + 1)] = win[128 * t:128 * (t + 1), 1:WP + 1]
        mmul = np.ones((MC, K + 1), dtype=np.float32)
        madd = np.full((MC, K + 1), const_add, dtype=np.float32)
        mmul[:, K] = 0.0
        madd[:, K] = 0.0
        if core == 0:
            for i in range(min(K, MC)):
                mmul[i, :K - i] = 0.0
                madd[i, :K - i] = np.float32(-1e9)
        maskp = np.concatenate(
            [mmul[0:128], madd[0:128], mmul[128:256], madd[128:256]],
            axis=1)
        in_maps.append({"xt2": np.ascontiguousarray(
                            np.concatenate([xtp, xtop], axis=1)),
                        "mask": np.ascontiguousarray(maskp), **shared})
    return in_maps, nn, nm


def _run(inputs, trace=False):
    assert int(np.asarray(inputs["K"])) == K
    in_maps, nn, nm = _prep_inputs(inputs)
    key = (nn, nm)
    if _cache.get("key") != key:
        _cache["nc"] = _build(nn, nm)
        _cache["key"] = key
    res = run_bass_kernel_spmd(_cache["nc"], in_maps, list(range(NCORES)),
                               trace=trace)
    out = np.concatenate([res.results[c]["out"] for c in range(NCORES)], axis=0)
    return out.astype(np.float32), res


def kernel(**inputs) -> np.ndarray:
    out, _ = _run(inputs, trace=False)
    return out
